# revision 1
# baseline (speedup 1.0000x reference)
"""DepthwiseSeparableDCNv2 for Trainium2 — self-contained 8-core SPMD Bass kernel.

kernel(**inputs) takes the full unsharded inputs and returns the full
[4, 256, 128, 128] float32 output. Sharding: 4 batch samples x 2 H-halves.
See _kernel() for the per-core pipeline.
"""
import numpy as np
import ml_dtypes
from contextlib import ExitStack

import concourse.bass as bass
from concourse import bacc
import concourse.mybir as mybir
from concourse.tile import TileContext
from concourse._compat import with_exitstack
from concourse import library_config

DT = mybir.dt
Alu = mybir.AluOpType
AF = mybir.ActivationFunctionType

B, C, H, W, O = 4, 128, 128, 128, 256
K2 = 9
ROWS = 64          # output rows per core
RB = 32            # idx-math batch rows
GG = 4             # rows per gather group
NG = RB // GG      # gather groups per batch
NIDX = GG * 2 * 128  # indices per gather instruction
IMG_U = 16385      # gather image units (16384 + 1 pad column)

CONS_W = 9 + 9 + 64 + 64 + 64 + 2 + 27  # 239

# scratch slot ids in the consolidated [128, NS, RB, 9] f32 tile
(S_MSK, S_WY, S_Y0S, S_Y1S, S_V0, S_V1, S_Y0C, S_Y1C, S_WX, S_X0S, S_X1S,
 S_XB, S_XB1, S_AS0, S_AS1, S_T0, S_T1, S_AWX, S_AWY, S_WY0M, S_WY1M,
 S_TMP) = range(22)
NS = 22
S_TYS = S_TMP   # tys -> txs -> adr share one slot (sequential lifetimes)
S_TXS = S_TMP
S_ADR = S_TMP
S_I0F = S_V0    # v0/v1 dead once wy0m/wy1m built
S_I1F = S_V1


def build_nc():
    nc = bacc.Bacc("TRN2", target_bir_lowering=False, debug=False,
                   num_devices=8, num_swdge_queues=4)
    xc = nc.dram_tensor("xc", [128, 66 * 130], DT.float16, kind="ExternalInput")
    xg = nc.dram_tensor("xg", [K2, IMG_U, 128], DT.float16, kind="ExternalInput")
    woff = nc.dram_tensor("woff", [128, K2 * 27], DT.float16, kind="ExternalInput")
    wpw = nc.dram_tensor("wpw", [128, 256], DT.float16, kind="ExternalInput")
    idn = nc.dram_tensor("idn", [128, 128], DT.float16, kind="ExternalInput")
    cons = nc.dram_tensor("cons", [128, CONS_W], DT.float32, kind="ExternalInput")
    out = nc.dram_tensor("out", [256, ROWS, 128], DT.float32, kind="ExternalOutput")

    with TileContext(nc) as tc:
        _kernel(tc, xc, xg, woff, wpw, idn, cons, out)

    nc.compile()
    legalize_single_wait(nc)
    bass.Bass.finalize(nc)
    return nc


@with_exitstack
def _kernel(ctx: ExitStack, tc: TileContext, xc, xg, woff, wpw, idn, cons, out):
    nc = tc.nc

    cpool = ctx.enter_context(tc.tile_pool(name="const", bufs=1))
    XC = cpool.tile([128, 66 * 130], DT.float16)
    nc.sync.dma_start(XC[:], xc.ap())
    WOF = cpool.tile([128, K2, 27], DT.float16)
    nc.sync.dma_start(WOF[:], woff.ap())
    WPW = cpool.tile([128, 256], DT.float16)
    nc.sync.dma_start(WPW[:], wpw.ap())
    IDN = cpool.tile([128, 128], DT.float16)
    nc.sync.dma_start(IDN[:], idn.ap())
    CON = cpool.tile([128, CONS_W], DT.float32)
    nc.sync.dma_start(CON[:], cons.ap())

    KY = CON[:, 0:9]           # ky + 16                  [128, 9]
    KX = CON[:, 9:18]          # w + kx + 16              [128, 9]
    HL = CON[:, 18:82]         # 16 - h   per row         [128, 64]
    HH = CON[:, 82:146]        # 143 - h  per row         [128, 64]
    HOF = CON[:, 146:210]      # 128*h - 2064 per row     [128, 64]
    BPW = CON[:, 210:212]      # fused pointwise bias     [128, 2]

    om_ps = ctx.enter_context(tc.tile_pool(name="omp", bufs=2, space="PSUM"))
    tr_ps = ctx.enter_context(tc.tile_pool(name="trp", bufs=2, space="PSUM"))
    pw_ps = ctx.enter_context(tc.tile_pool(name="pwp", bufs=2, space="PSUM"))
    oms_pool = ctx.enter_context(tc.tile_pool(name="oms", bufs=1))
    mpool = ctx.enter_context(tc.tile_pool(name="m", bufs=1))
    wpool = ctx.enter_context(tc.tile_pool(name="wp", bufs=2))
    ipool = ctx.enter_context(tc.tile_pool(name="ip", bufs=1))
    gpool = ctx.enter_context(tc.tile_pool(name="g", bufs=2))
    apool = ctx.enter_context(tc.tile_pool(name="acc", bufs=2))
    opool = ctx.enter_context(tc.tile_pool(name="o", bufs=2))

    out_v = out.ap().rearrange("(oh o) r w -> o oh r w", oh=2)
    gidx = [0]
    nidx_reg = ctx.enter_context(nc.gpsimd.register("nidx"))
    nc.gpsimd.reg_mov(nidx_reg, NIDX)

    for bt in range(2):
        # ---- offset conv: om.T [w, 27] per row ----
        OMS = oms_pool.tile([128, RB, 27], DT.float32, tag="oms")
        for r in range(RB):
            om = om_ps.tile([128, 27], DT.float32, tag="om", name="om")
            pos = (bt * RB + r + 1) * 130 + 1
            for t in range(K2):
                ty, tx = t // 3, t % 3
                sh = (ty - 1) * 130 + (tx - 1)
                nc.tensor.matmul(om[:], XC[:, pos + sh: pos + sh + 128],
                                 WOF[:, t, :], start=(t == 0), stop=(t == 8))
            nc.scalar.activation(OMS[:, r, :], om[:], AF.Copy)
        # b_off (broadcast over rows)
        _bof = CON[:, 212:239]
        bof_b = bass.AP(tensor=_bof.tensor, offset=_bof.offset,
                        ap=[list(_bof.ap[0]), [0, RB], [1, 27]])
        nc.vector.tensor_tensor(OMS[:], OMS[:], bof_b, op=Alu.add)

        # ---- index / weight math ----
        SCR = mpool.tile([128, NS, RB, K2], DT.float32, tag="scr", name="scr")

        def s(i):
            return SCR[:, i]

        nc.scalar.activation(s(S_MSK), OMS[:, :, 18:27], AF.Sigmoid)

        offs = OMS[:, :, 0:18].rearrange("p r (k two) -> p two r k", two=2)
        dy, dx = offs[:, 0], offs[:, 1]

        def bc9(ap128x9):   # [128, 9] -> [128, RB, 9] broadcast over rows
            return bass.AP(tensor=ap128x9.tensor, offset=ap128x9.offset,
                           ap=[list(ap128x9.ap[0]), [0, RB], [1, 9]])

        def bcrow(ap128x64):  # [128, 64] row-consts -> [128, RB, 9] for batch bt
            sl = ap128x64[:, bt * RB:(bt + 1) * RB]
            return bass.AP(tensor=sl.tensor, offset=sl.offset,
                           ap=[list(sl.ap[0]), [1, RB], [0, 9]])

        KYb, KXb = bc9(KY), bc9(KX)
        HLb, HHb, HOFb = bcrow(HL), bcrow(HH), bcrow(HOF)
        v = nc.vector

        W4 = wpool.tile([128, 4, RB, K2], DT.float32, tag="w4")
        IAL = ipool.tile([128, K2, NG, 2, GG], DT.int16, tag="ial")
        WR = wpool.tile([128, K2, NG, 2, GG, 8], DT.int16, tag="wr")

        v.tensor_tensor(s(S_TYS), dy, KYb, op=Alu.add)
        v.tensor_scalar(s(S_TYS), s(S_TYS), 0.0, None, Alu.max)
        # floor via the 2^23 magic number: RNE(x - 0.5) == floor(x) up to
        # integer ties, which bilinear continuity makes harmless
        v.tensor_scalar(s(S_Y0S), s(S_TYS), 8388607.5, 8388608.0,
                        Alu.add, Alu.subtract)
        v.tensor_tensor(s(S_WY), s(S_TYS), s(S_Y0S), op=Alu.subtract)
        v.tensor_scalar(s(S_Y1S), s(S_Y0S), 1.0, None, Alu.add)
        v.tensor_tensor(s(S_T0), s(S_Y0S), HLb, op=Alu.is_ge)
        v.tensor_tensor(s(S_T1), s(S_Y0S), HHb, op=Alu.is_le)
        v.tensor_tensor(s(S_V0), s(S_T0), s(S_T1), op=Alu.mult)
        v.tensor_tensor(s(S_T0), s(S_Y1S), HLb, op=Alu.is_ge)
        v.tensor_tensor(s(S_T1), s(S_Y1S), HHb, op=Alu.is_le)
        v.tensor_tensor(s(S_V1), s(S_T0), s(S_T1), op=Alu.mult)
        v.tensor_tensor(s(S_Y0C), s(S_Y0S), HLb, op=Alu.max)
        v.tensor_tensor(s(S_Y0C), s(S_Y0C), HHb, op=Alu.min)
        v.tensor_tensor(s(S_Y1C), s(S_Y1S), HLb, op=Alu.max)
        v.tensor_tensor(s(S_Y1C), s(S_Y1C), HHb, op=Alu.min)

        v.tensor_tensor(s(S_TXS), dx, KXb, op=Alu.add)
        v.tensor_scalar(s(S_TXS), s(S_TXS), 0.0, None, Alu.max)
        v.tensor_scalar(s(S_X0S), s(S_TXS), 8388607.5, 8388608.0,
                        Alu.add, Alu.subtract)
        v.tensor_tensor(s(S_WX), s(S_TXS), s(S_X0S), op=Alu.subtract)
        v.tensor_scalar(s(S_X1S), s(S_X0S), 1.0, None, Alu.add)
        v.tensor_scalar(s(S_XB), s(S_X0S), 16.0, None, Alu.max)
        v.tensor_scalar(s(S_XB), s(S_XB), 142.0, None, Alu.min)
        v.tensor_scalar(s(S_XB1), s(S_XB), 1.0, None, Alu.add)
        # slot weights: as_m = (1-wx)*[x0==xb+m] + wx*[x1==xb+m]
        v.tensor_scalar(s(S_AWX), s(S_WX), -1.0, 1.0, Alu.mult, Alu.add)
        v.tensor_tensor(s(S_T0), s(S_X0S), s(S_XB), op=Alu.is_equal)
        v.tensor_tensor(s(S_T1), s(S_X1S), s(S_XB), op=Alu.is_equal)
        v.tensor_tensor(s(S_T0), s(S_AWX), s(S_T0), op=Alu.mult)
        v.tensor_tensor(s(S_T1), s(S_WX), s(S_T1), op=Alu.mult)
        v.tensor_tensor(s(S_AS0), s(S_T0), s(S_T1), op=Alu.add)
        v.tensor_tensor(s(S_T0), s(S_X0S), s(S_XB1), op=Alu.is_equal)
        v.tensor_tensor(s(S_T1), s(S_X1S), s(S_XB1), op=Alu.is_equal)
        v.tensor_tensor(s(S_T0), s(S_AWX), s(S_T0), op=Alu.mult)
        v.tensor_tensor(s(S_T1), s(S_WX), s(S_T1), op=Alu.mult)
        v.tensor_tensor(s(S_AS1), s(S_T0), s(S_T1), op=Alu.add)
        # y weights with validity and mask folded in
        v.tensor_scalar(s(S_AWY), s(S_WY), -1.0, 1.0, Alu.mult, Alu.add)
        v.tensor_tensor(s(S_WY0M), s(S_AWY), s(S_V0), op=Alu.mult)
        v.tensor_tensor(s(S_WY0M), s(S_WY0M), s(S_MSK), op=Alu.mult)
        v.tensor_tensor(s(S_WY1M), s(S_WY), s(S_V1), op=Alu.mult)
        v.tensor_tensor(s(S_WY1M), s(S_WY1M), s(S_MSK), op=Alu.mult)
        v.tensor_tensor(W4[:, 0], s(S_WY0M), s(S_AS0), op=Alu.mult)
        v.tensor_tensor(W4[:, 1], s(S_WY0M), s(S_AS1), op=Alu.mult)
        v.tensor_tensor(W4[:, 2], s(S_WY1M), s(S_AS0), op=Alu.mult)
        v.tensor_tensor(W4[:, 3], s(S_WY1M), s(S_AS1), op=Alu.mult)
        # gather unit index = y0c*128 + (xb + 128*h - 2064)
        v.tensor_tensor(s(S_ADR), s(S_XB), HOFb, op=Alu.add)
        v.scalar_tensor_tensor(s(S_I0F), s(S_Y0C), 128.0, s(S_ADR),
                               Alu.mult, Alu.add)
        v.scalar_tensor_tensor(s(S_I1F), s(S_Y1C), 128.0, s(S_ADR),
                               Alu.mult, Alu.add)
        i0v = s(S_I0F).rearrange("p (g r) k -> p g r k", r=GG)
        i1v = s(S_I1F).rearrange("p (g r) k -> p g r k", r=GG)
        v.tensor_copy(IAL[:, :, :, 0, :].rearrange("p k g r -> p g r k"), i0v)
        v.tensor_copy(IAL[:, :, :, 1, :].rearrange("p k g r -> p g r k"), i1v)

        # ---- wrap indices into the 16-partition gather layout + replicate ----
        for sw in range(8):
            src = IAL[16 * sw:16 * (sw + 1)].rearrange("p k g c r -> p (k g c r)")
            nc.sync.dma_start(WR[0:16, :, :, :, :, sw], src)
        for gc in range(1, 8):
            nc.sync.dma_start(WR[16 * gc:16 * (gc + 1)], WR[0:16])

        # ---- gather + MAC + pointwise per gather group ----
        for gg in range(NG):
            GT = [gpool.tile([128, 2 * GG, 256], DT.float16, tag=f"gt{k}",
                             name=f"gt{k}") for k in range(K2)]
            for k in range(K2):
                src = bass.AP(tensor=xg, offset=k * IMG_U * 128,
                              ap=[[128, 16384], [1, 256]])
                idxs = WR[:, k, gg].rearrange("p c r s -> p (c r s)")
                nc.gpsimd.dma_gather(GT[k][:], src, idxs, NIDX, nidx_reg, 256,
                                     elem_step=128, queue_num=gidx[0] % 4)
                gidx[0] += 1
            OUTS = opool.tile([128, 2, GG, 128], DT.float32, tag="outs")
            for rr in range(GG):
                rb = gg * GG + rr
                ACC = apool.tile([128, 128], DT.float16, tag="acc")
                first = True
                for k in range(K2):
                    for c2 in range(2):
                        for s2 in range(2):
                            g = GT[k][:, c2 * GG + rr, s2 * 128:(s2 + 1) * 128]
                            wsc = W4[:, c2 * 2 + s2, rb, k:k + 1]
                            if first:
                                v.tensor_scalar(ACC[:], g, wsc, None, Alu.mult)
                                first = False
                            else:
                                v.scalar_tensor_tensor(ACC[:], g, wsc, ACC[:],
                                                       Alu.mult, Alu.add)
                TR = tr_ps.tile([128, 128], DT.float16, tag="tr", name="tr")
                nc.tensor.transpose(TR[:], ACC[:], IDN[:])
                RT = apool.tile([128, 128], DT.float16, tag="rt")
                nc.scalar.activation(RT[:], TR[:], AF.Copy)
                for oh in range(2):
                    PW = pw_ps.tile([128, 128], DT.float32, tag=f"pw{oh}",
                                    name=f"pw{oh}")
                    nc.tensor.matmul(PW[:], WPW[:, oh * 128:(oh + 1) * 128],
                                     RT[:], start=True, stop=True)
                    nc.scalar.activation(OUTS[:, oh, rr, :], PW[:], AF.Identity,
                                         bias=BPW[:, oh:oh + 1])
            r0 = bt * RB + gg * GG
            nc.sync.dma_start(out_v[:, :, r0:r0 + GG, :], OUTS[:])


# ---------------- host side ----------------

def host_prep(inputs):
    x = np.asarray(inputs["x"], np.float32)
    w_off = np.asarray(inputs["w_off"], np.float32)
    b_off = np.asarray(inputs["b_off"], np.float32)
    w_dw = np.asarray(inputs["w_dw"], np.float32)
    b_dw = np.asarray(inputs["b_dw"], np.float32)
    w_pw = np.asarray(inputs["w_pw"], np.float32)
    b_pw = np.asarray(inputs["b_pw"], np.float32)

    wk = w_dw.reshape(C, K2)
    woff_p = np.ascontiguousarray(
        w_off.transpose(1, 2, 3, 0).reshape(C, K2 * 27)).astype(np.float16)
    wpw_p = np.ascontiguousarray(w_pw.T).astype(np.float16)
    idn = np.eye(128, dtype=np.float16)
    bpw_eff = (b_pw + w_pw @ b_dw).astype(np.float32)

    ky = (np.arange(K2) // 3 - 1).astype(np.float32)
    kx = (np.arange(K2) % 3 - 1).astype(np.float32)

    xgs = []
    for b in range(B):
        flat = np.ascontiguousarray(x[b].transpose(1, 2, 0).reshape(H * W, C))
        img = np.zeros([K2, IMG_U, 128], np.float16)
        for k in range(K2):
            img[k, :H * W] = flat * wk[:, k][None, :]
        xgs.append(img)

    in_maps = []
    for core in range(8):
        b, half = core // 2, core % 2
        r0 = half * ROWS
        xcp = np.zeros([C, 66, 130], np.float32)
        lo, hi = max(r0 - 1, 0), min(r0 + 65, H)
        xcp[:, lo - (r0 - 1): hi - (r0 - 1), 1:129] = x[b][:, lo:hi, :]
        xcp = xcp.astype(np.float16).reshape(C, 66 * 130)

        hvec = (r0 + np.arange(ROWS)).astype(np.float32)
        cons = np.zeros([128, CONS_W], np.float32)
        cons[:, 0:9] = ky[None, :] + 16.0
        cons[:, 9:18] = kx[None, :] + 16.0 + np.arange(128, dtype=np.float32)[:, None]
        cons[:, 18:82] = (16.0 - hvec)[None, :]
        cons[:, 82:146] = (143.0 - hvec)[None, :]
        cons[:, 146:210] = (128.0 * hvec - 2064.0)[None, :]
        cons[:, 210:212] = bpw_eff.reshape(2, 128).T
        cons[:, 212:239] = b_off[None, :]

        in_maps.append({
            "xc": xcp, "xg": xgs[b], "woff": woff_p, "wpw": wpw_p,
            "idn": idn, "cons": cons,
        })
    return in_maps


def assemble(results):
    out = np.zeros([B, O, H, W], np.float32)
    for core, r in enumerate(results):
        b, half = core // 2, core % 2
        out[b, :, half * ROWS:(half + 1) * ROWS, :] = r["out"]
    return out


# ---- single-sync-wait legalization (inlined) ----
_doc = """Legalize BIR for walrus builds that allow only ONE sync wait per
instruction: hoist extra waits onto same-engine NOPs inserted immediately
before the offending instruction."""
import copy

def _make_nop(nc, engine):
    nop = nc.engines[engine].nop(nofuse=True).ins
    # the builder appended it to nc.cur_bb; steal it from wherever it landed
    for f in nc.m.functions:
        for bb in f.blocks:
            il = bb.instructions
            if il and il[-1].name == nop.name:
                il.pop()
                bb.instructions = il
                return nop
    raise RuntimeError("freshly built nop not found")

def legalize_single_wait(nc):
    n_split = 0
    for f in nc.m.functions:
        for bb in f.blocks:
            insts = bb.instructions
            if not any(i.sync_info and len(i.sync_info.on_wait) > 1 for i in insts):
                continue
            out = []
            for inst in insts:
                si = inst.sync_info
                if si and len(si.on_wait) > 1:
                    waits = list(si.on_wait)
                    for w in waits[:-1]:
                        nop = _make_nop(nc, inst.engine)
                        nsi = copy.deepcopy(si)
                        nsi.on_wait = [w]
                        nsi.on_update = []
                        nop.sync_info = nsi
                        out.append(nop)
                    si.on_wait = [waits[-1]]
                    n_split += 1
                out.append(inst)
            bb.instructions = out
    return n_split


_CACHED_NC = None


def kernel(**inputs):
    global _CACHED_NC
    from concourse import bass_utils
    in_maps = host_prep(inputs)
    if _CACHED_NC is None:
        _CACHED_NC = build_nc()
    res = bass_utils.run_bass_kernel_spmd(_CACHED_NC, in_maps,
                                          core_ids=list(range(8)))
    return assemble(res.results)



# revision 39
# speedup vs baseline: 1.1469x; 1.1469x over previous
"""DepthwiseSeparableDCNv2 for Trainium2 — self-contained 8-core SPMD Bass kernel.

kernel(**inputs) takes the full unsharded inputs and returns the full
[4, 256, 128, 128] float32 output. Sharding: 4 batch samples x 2 H-halves.
See _kernel() for the per-core pipeline.

The bilinear MAC runs on the PE array as weighted transposes:
TR[c, x] += sum_x' G_j[x', c] * D_j[x', x] with D_j = diag(w_j) built on
DVE via tensor_scalar from the identity (4x DVE mode, 93ns vs 194ns for
scalar_tensor_tensor). The 36 terms per output row accumulate in fp32
PSUM. Gathers use NIDX=4096 to amortize the 994ns SWDGE fixed overhead.
"""
import numpy as np
import ml_dtypes
from contextlib import ExitStack

import concourse.bass as bass
from concourse import bacc
import concourse.mybir as mybir
from concourse.tile import TileContext
from concourse._compat import with_exitstack
from concourse import library_config

DT = mybir.dt
Alu = mybir.AluOpType
AF = mybir.ActivationFunctionType

B, C, H, W, O = 4, 128, 128, 128, 256
K2 = 9
ROWS = 64          # output rows per core
RB = 16            # idx-math batch rows
GG = 4             # rows per gather group
NG = RB // GG      # gather groups per batch
NIDX = GG * 2 * 128  # indices per gather instruction
IMG_U = 16385      # gather image units (16384 + 1 pad column)

CONS_W = 9 + 9 + 64 + 64 + 64 + 2 + 27  # 239

# scratch slot ids in the consolidated [128, NS, RB, 9] f32 tile
(S_MSK, S_WY, S_Y0S, S_Y1S, S_V0, S_V1, S_Y0C, S_Y1C, S_WX, S_X0S, S_X1S,
 S_XB, S_XB1, S_AS0, S_AS1, S_T0, S_T1, S_AWX, S_AWY, S_WY0M, S_WY1M,
 S_TMP) = range(22)
NS = 22
S_TYS = S_TMP   # tys -> txs -> adr share one slot (sequential lifetimes)
S_TXS = S_TMP
S_ADR = S_TMP
S_I0F = S_V0    # v0/v1 dead once wy0m/wy1m built
S_I1F = S_V1


def build_nc():
    nc = bacc.Bacc("TRN2", target_bir_lowering=False, debug=False,
                   num_devices=8, num_swdge_queues=4)
    xc = nc.dram_tensor("xc", [128, 66 * 130], DT.float16, kind="ExternalInput")
    xg = nc.dram_tensor("xg", [K2, IMG_U, 128], DT.float16, kind="ExternalInput")
    woff = nc.dram_tensor("woff", [128, K2 * 27], DT.float16, kind="ExternalInput")
    wpw = nc.dram_tensor("wpw", [128, 256], DT.float16, kind="ExternalInput")
    idn = nc.dram_tensor("idn", [128, 128], DT.float16, kind="ExternalInput")
    idnp = nc.dram_tensor("idnp", [128, 128], DT.float32, kind="ExternalInput")
    cons = nc.dram_tensor("cons", [128, CONS_W], DT.float32, kind="ExternalInput")
    out = nc.dram_tensor("out", [256, ROWS, 128], DT.float16, kind="ExternalOutput")

    with TileContext(nc) as tc:
        _kernel(tc, xc, xg, woff, wpw, idn, idnp, cons, out)

    nc.compile()
    legalize_single_wait(nc)
    bass.Bass.finalize(nc)
    return nc


@with_exitstack
def _kernel(ctx: ExitStack, tc: TileContext, xc, xg, woff, wpw, idn, idnp, cons,
            out):
    nc = tc.nc

    cpool = ctx.enter_context(tc.tile_pool(name="const", bufs=1))
    XC = cpool.tile([128, 66 * 130], DT.float16)
    nc.sync.dma_start(XC[:], xc.ap())
    WOF = cpool.tile([128, K2, 27], DT.float16)
    nc.sync.dma_start(WOF[:], woff.ap())
    WPW = cpool.tile([128, 256], DT.float16)
    nc.sync.dma_start(WPW[:], wpw.ap())
    IDN = cpool.tile([128, 128], DT.float16)
    nc.sync.dma_start(IDN[:], idn.ap())
    IDNP = cpool.tile([128, 128], DT.float32)
    nc.sync.dma_start(IDNP[:], idnp.ap())
    CON = cpool.tile([128, CONS_W], DT.float32)
    nc.sync.dma_start(CON[:], cons.ap())

    KY = CON[:, 0:9]           # ky + 16                  [128, 9]
    KX = CON[:, 9:18]          # w + kx + 16              [128, 9]
    HL = CON[:, 18:82]         # 16 - h   per row         [128, 64]
    HH = CON[:, 82:146]        # 143 - h  per row         [128, 64]
    HOF = CON[:, 146:210]      # 128*h - 2064 per row     [128, 64]
    BPW = CON[:, 210:212]      # fused pointwise bias     [128, 2]

    om_ps = ctx.enter_context(tc.tile_pool(name="omp", bufs=2, space="PSUM"))
    pw_ps = ctx.enter_context(tc.tile_pool(name="pwp", bufs=1, space="PSUM"))
    tp_ps = ctx.enter_context(tc.tile_pool(name="tpp", bufs=1, space="PSUM"))
    tpool = ctx.enter_context(tc.tile_pool(name="t3", bufs=2))
    ipool = ctx.enter_context(tc.tile_pool(name="ip", bufs=1))
    oms_pool = ctx.enter_context(tc.tile_pool(name="oms", bufs=1))
    mpool = ctx.enter_context(tc.tile_pool(name="m", bufs=1))
    wpool = ctx.enter_context(tc.tile_pool(name="wp", bufs=2))
    gpool = ctx.enter_context(tc.tile_pool(name="g", bufs=3))
    dpool = ctx.enter_context(tc.tile_pool(name="d", bufs=3))
    rpool = ctx.enter_context(tc.tile_pool(name="rt", bufs=2))
    opool = ctx.enter_context(tc.tile_pool(name="o", bufs=2))
    trs_ps = ctx.enter_context(tc.tile_pool(name="trs", bufs=2, space="PSUM"))

    out_v = out.ap().rearrange("(oh o) r w -> o oh r w", oh=2)
    gidx = [0]
    nidx_reg = ctx.enter_context(nc.gpsimd.register("nidx"))
    nc.gpsimd.reg_mov(nidx_reg, NIDX)

    for bt in range(ROWS // RB):
        # ---- offset conv: om.T [w, 27] per row ----
        OMS = oms_pool.tile([128, RB, 27], DT.float32, tag="oms")
        for r in range(RB):
            om = om_ps.tile([128, 27], DT.float32, tag="om", name="om")
            pos = (bt * RB + r + 1) * 130 + 1
            for t in range(K2):
                ty, tx = t // 3, t % 3
                sh = (ty - 1) * 130 + (tx - 1)
                nc.tensor.matmul(om[:], XC[:, pos + sh: pos + sh + 128],
                                 WOF[:, t, :], start=(t == 0), stop=(t == 8))
            nc.scalar.activation(OMS[:, r, :], om[:], AF.Copy)
        # b_off (broadcast over rows)
        _bof = CON[:, 212:239]
        bof_b = bass.AP(tensor=_bof.tensor, offset=_bof.offset,
                        ap=[list(_bof.ap[0]), [0, RB], [1, 27]])
        nc.vector.tensor_tensor(OMS[:], OMS[:], bof_b, op=Alu.add)

        # ---- index / weight math ----
        SCR = mpool.tile([128, NS, RB, K2], DT.float32, tag="scr", name="scr")

        def s(i):
            return SCR[:, i]

        nc.scalar.activation(s(S_MSK), OMS[:, :, 18:27], AF.Sigmoid)

        offs = OMS[:, :, 0:18].rearrange("p r (k two) -> p two r k", two=2)
        dy, dx = offs[:, 0], offs[:, 1]

        def bc9(ap128x9):   # [128, 9] -> [128, RB, 9] broadcast over rows
            return bass.AP(tensor=ap128x9.tensor, offset=ap128x9.offset,
                           ap=[list(ap128x9.ap[0]), [0, RB], [1, 9]])

        def bcrow(ap128x64):  # [128, 64] row-consts -> [128, RB, 9] for batch bt
            sl = ap128x64[:, bt * RB:(bt + 1) * RB]
            return bass.AP(tensor=sl.tensor, offset=sl.offset,
                           ap=[list(sl.ap[0]), [1, RB], [0, 9]])

        KYb, KXb = bc9(KY), bc9(KX)
        HLb, HHb, HOFb = bcrow(HL), bcrow(HH), bcrow(HOF)
        v = nc.vector

        W4 = wpool.tile([128, 4, RB, K2], DT.float32, tag="w4")
        WR = wpool.tile([128, K2, NG, 2, GG, 8], DT.int16, tag="wr")
        IAL = ipool.tile([128, K2, NG, 2, GG], DT.int16, tag="ial")

        v.tensor_tensor(s(S_TYS), dy, KYb, op=Alu.add)
        v.tensor_scalar(s(S_TYS), s(S_TYS), 0.0, None, Alu.max)
        # floor via the 2^23 magic number: RNE(x - 0.5) == floor(x) up to
        # integer ties, which bilinear continuity makes harmless
        v.tensor_scalar(s(S_Y0S), s(S_TYS), 8388607.5, 8388608.0,
                        Alu.add, Alu.subtract)
        v.tensor_tensor(s(S_WY), s(S_TYS), s(S_Y0S), op=Alu.subtract)
        v.tensor_scalar(s(S_Y1S), s(S_Y0S), 1.0, None, Alu.add)
        v.tensor_tensor(s(S_T0), s(S_Y0S), HLb, op=Alu.is_ge)
        v.tensor_tensor(s(S_T1), s(S_Y0S), HHb, op=Alu.is_le)
        v.tensor_tensor(s(S_V0), s(S_T0), s(S_T1), op=Alu.mult)
        v.tensor_tensor(s(S_T0), s(S_Y1S), HLb, op=Alu.is_ge)
        v.tensor_tensor(s(S_T1), s(S_Y1S), HHb, op=Alu.is_le)
        v.tensor_tensor(s(S_V1), s(S_T0), s(S_T1), op=Alu.mult)
        v.tensor_tensor(s(S_Y0C), s(S_Y0S), HLb, op=Alu.max)
        v.tensor_tensor(s(S_Y0C), s(S_Y0C), HHb, op=Alu.min)
        v.tensor_tensor(s(S_Y1C), s(S_Y1S), HLb, op=Alu.max)
        v.tensor_tensor(s(S_Y1C), s(S_Y1C), HHb, op=Alu.min)

        v.tensor_tensor(s(S_TXS), dx, KXb, op=Alu.add)
        v.tensor_scalar(s(S_TXS), s(S_TXS), 0.0, None, Alu.max)
        v.tensor_scalar(s(S_X0S), s(S_TXS), 8388607.5, 8388608.0,
                        Alu.add, Alu.subtract)
        v.tensor_tensor(s(S_WX), s(S_TXS), s(S_X0S), op=Alu.subtract)
        v.tensor_scalar(s(S_X1S), s(S_X0S), 1.0, None, Alu.add)
        v.tensor_scalar(s(S_XB), s(S_X0S), 16.0, None, Alu.max)
        v.tensor_scalar(s(S_XB), s(S_XB), 142.0, None, Alu.min)
        v.tensor_scalar(s(S_XB1), s(S_XB), 1.0, None, Alu.add)
        # slot weights: as_m = (1-wx)*[x0==xb+m] + wx*[x1==xb+m]
        v.tensor_scalar(s(S_AWX), s(S_WX), -1.0, 1.0, Alu.mult, Alu.add)
        v.tensor_tensor(s(S_T0), s(S_X0S), s(S_XB), op=Alu.is_equal)
        v.tensor_tensor(s(S_T1), s(S_X1S), s(S_XB), op=Alu.is_equal)
        v.tensor_tensor(s(S_T0), s(S_AWX), s(S_T0), op=Alu.mult)
        v.tensor_tensor(s(S_T1), s(S_WX), s(S_T1), op=Alu.mult)
        v.tensor_tensor(s(S_AS0), s(S_T0), s(S_T1), op=Alu.add)
        v.tensor_tensor(s(S_T0), s(S_X0S), s(S_XB1), op=Alu.is_equal)
        v.tensor_tensor(s(S_T1), s(S_X1S), s(S_XB1), op=Alu.is_equal)
        v.tensor_tensor(s(S_T0), s(S_AWX), s(S_T0), op=Alu.mult)
        v.tensor_tensor(s(S_T1), s(S_WX), s(S_T1), op=Alu.mult)
        v.tensor_tensor(s(S_AS1), s(S_T0), s(S_T1), op=Alu.add)
        # y weights with validity and mask folded in
        v.tensor_scalar(s(S_AWY), s(S_WY), -1.0, 1.0, Alu.mult, Alu.add)
        v.tensor_tensor(s(S_WY0M), s(S_AWY), s(S_V0), op=Alu.mult)
        v.tensor_tensor(s(S_WY0M), s(S_WY0M), s(S_MSK), op=Alu.mult)
        v.tensor_tensor(s(S_WY1M), s(S_WY), s(S_V1), op=Alu.mult)
        v.tensor_tensor(s(S_WY1M), s(S_WY1M), s(S_MSK), op=Alu.mult)
        v.tensor_tensor(W4[:, 0], s(S_WY0M), s(S_AS0), op=Alu.mult)
        v.tensor_tensor(W4[:, 1], s(S_WY0M), s(S_AS1), op=Alu.mult)
        v.tensor_tensor(W4[:, 2], s(S_WY1M), s(S_AS0), op=Alu.mult)
        v.tensor_tensor(W4[:, 3], s(S_WY1M), s(S_AS1), op=Alu.mult)
        # gather unit index = y0c*128 + (xb + 128*h - 2064)
        v.tensor_tensor(s(S_ADR), s(S_XB), HOFb, op=Alu.add)
        v.scalar_tensor_tensor(s(S_I0F), s(S_Y0C), 128.0, s(S_ADR),
                               Alu.mult, Alu.add)
        v.scalar_tensor_tensor(s(S_I1F), s(S_Y1C), 128.0, s(S_ADR),
                               Alu.mult, Alu.add)

        # ---- wrap indices into the 16-partition gather layout + replicate ----
        i0v = s(S_I0F).rearrange("p (g r) k -> p g r k", r=GG)
        i1v = s(S_I1F).rearrange("p (g r) k -> p g r k", r=GG)
        v.tensor_copy(IAL[:, :, :, 0, :].rearrange("p k g r -> p g r k"), i0v)
        v.tensor_copy(IAL[:, :, :, 1, :].rearrange("p k g r -> p g r k"), i1v)
        for sw in range(8):
            src = IAL[16 * sw:16 * (sw + 1)].rearrange("p k g c r -> p (k g c r)")
            nc.sync.dma_start(WR[0:16, :, :, :, :, sw], src)
        for gc in range(1, 8):
            nc.sync.dma_start(WR[16 * gc:16 * (gc + 1)], WR[0:16])

        # ---- gather + weighted-transpose MAC on PE + pointwise ----
        for gg in range(NG):
            TRS = trs_ps.tile([128, GG, 128], DT.float32, tag="trs",
                              name="trs")
            for k in range(K2):
                GT = gpool.tile([128, 2 * GG, 256], DT.float16, tag="gt",
                                name="gt")
                src = bass.AP(tensor=xg, offset=k * IMG_U * 128,
                              ap=[[128, 16384], [1, 256]])
                idxs = WR[:, k, gg].rearrange("p c r s -> p (c r s)")
                nc.gpsimd.dma_gather(GT[:], src, idxs, NIDX, nidx_reg, 256,
                                     elem_step=128, queue_num=gidx[0] % 4)
                gidx[0] += 1
                DK = dpool.tile([128, GG, 4, 128], DT.float16, tag="d")
                for rr in range(GG):
                    rb = gg * GG + rr
                    for c2 in range(2):
                        for s2 in range(2):
                            wsc = W4[:, c2 * 2 + s2, rb, k:k + 1]
                            v.tensor_scalar(DK[:, rr, c2 * 2 + s2], IDN[:],
                                            wsc, None, Alu.mult)
                for rr in range(GG):
                    for c2 in range(2):
                        for s2 in range(2):
                            g = GT[:, c2 * GG + rr, s2 * 128:(s2 + 1) * 128]
                            # psum zero regions are 2KB = 4 rows of TRS;
                            # one accumulation group per region
                            nc.tensor.matmul(
                                TRS[:, rr], g, DK[:, rr, c2 * 2 + s2],
                                start=(k == 0 and rr % 4 == 0 and c2 == 0
                                       and s2 == 0),
                                stop=(k == 8 and rr % 4 == 3 and c2 == 1
                                      and s2 == 1))
            OUTS = opool.tile([128, 2, GG, 128], DT.float16, tag="outs")
            for rr in range(GG):
                RT = rpool.tile([128, 128], DT.float16, tag="rt")
                nc.scalar.activation(RT[:], TRS[:, rr], AF.Copy)
                for oh in range(2):
                    PW = pw_ps.tile([128, 128], DT.float32, tag="pw",
                                    name="pw")
                    nc.tensor.matmul(PW[:], WPW[:, oh * 128:(oh + 1) * 128],
                                     RT[:], start=True, stop=True)
                    nc.scalar.activation(OUTS[:, oh, rr, :], PW[:], AF.Identity,
                                         bias=BPW[:, oh:oh + 1])
            r0 = bt * RB + gg * GG
            nc.sync.dma_start(out_v[:, :, r0:r0 + GG, :], OUTS[:])


# ---------------- host side ----------------

def host_prep(inputs):
    x = np.asarray(inputs["x"], np.float32)
    w_off = np.asarray(inputs["w_off"], np.float32)
    b_off = np.asarray(inputs["b_off"], np.float32)
    w_dw = np.asarray(inputs["w_dw"], np.float32)
    b_dw = np.asarray(inputs["b_dw"], np.float32)
    w_pw = np.asarray(inputs["w_pw"], np.float32)
    b_pw = np.asarray(inputs["b_pw"], np.float32)

    wk = w_dw.reshape(C, K2)
    woff_p = np.ascontiguousarray(
        w_off.transpose(1, 2, 3, 0).reshape(C, K2 * 27)).astype(np.float16)
    wpw_p = np.ascontiguousarray(w_pw.T).astype(np.float16)
    idn = np.eye(128, dtype=np.float16)
    idnp = np.zeros((128, 128), np.float32)
    jj = np.arange(128)
    idnp[(jj % 8) * 16 + jj // 8, jj] = 1.0
    bpw_eff = (b_pw + w_pw @ b_dw).astype(np.float32)

    ky = (np.arange(K2) // 3 - 1).astype(np.float32)
    kx = (np.arange(K2) % 3 - 1).astype(np.float32)

    xgs = []
    for b in range(B):
        flat = np.ascontiguousarray(x[b].transpose(1, 2, 0).reshape(H * W, C))
        img = np.zeros([K2, IMG_U, 128], np.float16)
        for k in range(K2):
            img[k, :H * W] = flat * wk[:, k][None, :]
        xgs.append(img)

    in_maps = []
    for core in range(8):
        b, half = core // 2, core % 2
        r0 = half * ROWS
        xcp = np.zeros([C, 66, 130], np.float32)
        lo, hi = max(r0 - 1, 0), min(r0 + 65, H)
        xcp[:, lo - (r0 - 1): hi - (r0 - 1), 1:129] = x[b][:, lo:hi, :]
        xcp = xcp.astype(np.float16).reshape(C, 66 * 130)

        hvec = (r0 + np.arange(ROWS)).astype(np.float32)
        cons = np.zeros([128, CONS_W], np.float32)
        cons[:, 0:9] = ky[None, :] + 16.0
        cons[:, 9:18] = kx[None, :] + 16.0 + np.arange(128, dtype=np.float32)[:, None]
        cons[:, 18:82] = (16.0 - hvec)[None, :]
        cons[:, 82:146] = (143.0 - hvec)[None, :]
        cons[:, 146:210] = (128.0 * hvec - 2064.0)[None, :]
        cons[:, 210:212] = bpw_eff.reshape(2, 128).T
        cons[:, 212:239] = b_off[None, :]

        in_maps.append({
            "xc": xcp, "xg": xgs[b], "woff": woff_p, "wpw": wpw_p,
            "idn": idn, "idnp": idnp, "cons": cons,
        })
    return in_maps


def assemble(results):
    out = np.zeros([B, O, H, W], np.float32)
    for core, r in enumerate(results):
        b, half = core // 2, core % 2
        out[b, :, half * ROWS:(half + 1) * ROWS, :] = r["out"].astype(np.float32)
    return out


# ---- single-sync-wait legalization (inlined) ----
_doc = """Legalize BIR for walrus builds that allow only ONE sync wait per
instruction: hoist extra waits onto same-engine NOPs inserted immediately
before the offending instruction."""
import copy

def _make_nop(nc, engine):
    nop = nc.engines[engine].nop(nofuse=True).ins
    # the builder appended it to nc.cur_bb; steal it from wherever it landed
    for f in nc.m.functions:
        for bb in f.blocks:
            il = bb.instructions
            if il and il[-1].name == nop.name:
                il.pop()
                bb.instructions = il
                return nop
    raise RuntimeError("freshly built nop not found")

def legalize_single_wait(nc):
    n_split = 0
    for f in nc.m.functions:
        for bb in f.blocks:
            insts = bb.instructions
            if not any(i.sync_info and len(i.sync_info.on_wait) > 1 for i in insts):
                continue
            out = []
            for inst in insts:
                si = inst.sync_info
                if si and len(si.on_wait) > 1:
                    waits = list(si.on_wait)
                    for w in waits[:-1]:
                        nop = _make_nop(nc, inst.engine)
                        nsi = copy.deepcopy(si)
                        nsi.on_wait = [w]
                        nsi.on_update = []
                        nop.sync_info = nsi
                        out.append(nop)
                    si.on_wait = [waits[-1]]
                    n_split += 1
                out.append(inst)
            bb.instructions = out
    return n_split


_CACHED_NC = None


def kernel(**inputs):
    global _CACHED_NC
    from concourse import bass_utils
    in_maps = host_prep(inputs)
    if _CACHED_NC is None:
        _CACHED_NC = build_nc()
    res = bass_utils.run_bass_kernel_spmd(_CACHED_NC, in_maps,
                                          core_ids=list(range(8)))
    return assemble(res.results)


# revision 42
# speedup vs baseline: 1.1893x; 1.0370x over previous
"""DepthwiseSeparableDCNv2 for Trainium2 — self-contained 8-core SPMD Bass kernel.

kernel(**inputs) takes the full unsharded inputs and returns the full
[4, 256, 128, 128] float32 output. Sharding: 4 batch samples x 2 H-halves.
See _kernel() for the per-core pipeline.

The bilinear MAC runs on the PE array as weighted transposes:
TR[c, x] += sum_x' G_j[x', c] * D_j[x', x] with D_j = diag(w_j) built on
DVE via tensor_scalar from the identity (4x DVE mode, 93ns vs 194ns for
scalar_tensor_tensor). The 36 terms per output row accumulate in fp32
PSUM. Gathers use NIDX=4096 to amortize the 994ns SWDGE fixed overhead.
"""
import numpy as np
import ml_dtypes
from contextlib import ExitStack

import concourse.bass as bass
from concourse import bacc
import concourse.mybir as mybir
from concourse.tile import TileContext
from concourse._compat import with_exitstack
from concourse import library_config

DT = mybir.dt
Alu = mybir.AluOpType
AF = mybir.ActivationFunctionType

B, C, H, W, O = 4, 128, 128, 128, 256
K2 = 9
ROWS = 64          # output rows per core
RB = 16            # idx-math batch rows
GG = 4             # rows per gather group
NG = RB // GG      # gather groups per batch
NIDX = GG * 2 * 128  # indices per gather instruction
IMG_U = 16385      # gather image units (16384 + 1 pad column)

CONS_W = 9 + 9 + 64 + 64 + 64 + 2 + 27  # 239

# scratch slot ids in the consolidated [128, NS, RB, 9] f32 tile
(S_MSK, S_WY, S_Y0S, S_Y1S, S_V0, S_V1, S_Y0C, S_Y1C, S_WX, S_X0S, S_X1S,
 S_XB, S_XB1, S_AS0, S_AS1, S_T0, S_T1, S_AWX, S_AWY, S_WY0M, S_WY1M,
 S_TMP) = range(22)
NS = 22
S_TYS = S_TMP   # tys -> txs -> adr share one slot (sequential lifetimes)
S_TXS = S_TMP
S_ADR = S_TMP
S_I0F = S_V0    # v0/v1 dead once wy0m/wy1m built
S_I1F = S_V1


def build_nc():
    nc = bacc.Bacc("TRN2", target_bir_lowering=False, debug=False,
                   num_devices=8, num_swdge_queues=4)
    xc = nc.dram_tensor("xc", [128, 66 * 130], DT.float16, kind="ExternalInput")
    xg = nc.dram_tensor("xg", [K2, IMG_U, 128], DT.float16, kind="ExternalInput")
    woff = nc.dram_tensor("woff", [128, K2 * 27], DT.float16, kind="ExternalInput")
    wpw = nc.dram_tensor("wpw", [128, 256], DT.float16, kind="ExternalInput")
    idn = nc.dram_tensor("idn", [128, 128], DT.float16, kind="ExternalInput")
    idnp = nc.dram_tensor("idnp", [128, 128], DT.float32, kind="ExternalInput")
    cons = nc.dram_tensor("cons", [128, CONS_W], DT.float32, kind="ExternalInput")
    out = nc.dram_tensor("out", [256, ROWS, 128], DT.float16, kind="ExternalOutput")
    wrscr = nc.dram_tensor("wrscr", [ROWS // 16, 128, 4, 128], DT.int16,
                           kind="Internal")
    wrscrb = nc.dram_tensor("wrscrb", [ROWS // 16, 16, 4, 128], DT.int16,
                            kind="Internal")

    with TileContext(nc) as tc:
        _kernel(tc, xc, xg, woff, wpw, idn, idnp, cons, out, wrscr, wrscrb)

    nc.compile()
    legalize_single_wait(nc)
    bass.Bass.finalize(nc)
    return nc


@with_exitstack
def _kernel(ctx: ExitStack, tc: TileContext, xc, xg, woff, wpw, idn, idnp, cons,
            out, wrscr, wrscrb):
    nc = tc.nc

    cpool = ctx.enter_context(tc.tile_pool(name="const", bufs=1))
    XC = cpool.tile([128, 66 * 130], DT.float16)
    nc.sync.dma_start(XC[:], xc.ap())
    WOF = cpool.tile([128, K2, 27], DT.float16)
    nc.sync.dma_start(WOF[:], woff.ap())
    WPW = cpool.tile([128, 256], DT.float16)
    nc.sync.dma_start(WPW[:], wpw.ap())
    IDN = cpool.tile([128, 128], DT.float16)
    nc.sync.dma_start(IDN[:], idn.ap())
    IDNP = cpool.tile([128, 128], DT.float32)
    nc.sync.dma_start(IDNP[:], idnp.ap())
    CON = cpool.tile([128, CONS_W], DT.float32)
    nc.sync.dma_start(CON[:], cons.ap())

    KY = CON[:, 0:9]           # ky + 16                  [128, 9]
    KX = CON[:, 9:18]          # w + kx + 16              [128, 9]
    HL = CON[:, 18:82]         # 16 - h   per row         [128, 64]
    HH = CON[:, 82:146]        # 143 - h  per row         [128, 64]
    HOF = CON[:, 146:210]      # 128*h - 2064 per row     [128, 64]
    BPW = CON[:, 210:212]      # fused pointwise bias     [128, 2]

    om_ps = ctx.enter_context(tc.tile_pool(name="omp", bufs=2, space="PSUM"))
    pw_ps = ctx.enter_context(tc.tile_pool(name="pwp", bufs=1, space="PSUM"))
    tp_ps = ctx.enter_context(tc.tile_pool(name="tpp", bufs=1, space="PSUM"))
    tpool = ctx.enter_context(tc.tile_pool(name="t3", bufs=2))
    oms_pool = ctx.enter_context(tc.tile_pool(name="oms", bufs=1))
    mpool = ctx.enter_context(tc.tile_pool(name="m", bufs=1))
    wpool = ctx.enter_context(tc.tile_pool(name="wp", bufs=2))
    gpool = ctx.enter_context(tc.tile_pool(name="g", bufs=3))
    dpool = ctx.enter_context(tc.tile_pool(name="d", bufs=3))
    rpool = ctx.enter_context(tc.tile_pool(name="rt", bufs=2))
    opool = ctx.enter_context(tc.tile_pool(name="o", bufs=2))
    trs_ps = ctx.enter_context(tc.tile_pool(name="trs", bufs=2, space="PSUM"))

    out_v = out.ap().rearrange("(oh o) r w -> o oh r w", oh=2)
    gidx = [0]
    nidx_reg = ctx.enter_context(nc.gpsimd.register("nidx"))
    nc.gpsimd.reg_mov(nidx_reg, NIDX)

    for bt in range(ROWS // RB):
        # ---- offset conv: om.T [w, 27] per row ----
        OMS = oms_pool.tile([128, RB, 27], DT.float32, tag="oms")
        for r in range(RB):
            om = om_ps.tile([128, 27], DT.float32, tag="om", name="om")
            pos = (bt * RB + r + 1) * 130 + 1
            for t in range(K2):
                ty, tx = t // 3, t % 3
                sh = (ty - 1) * 130 + (tx - 1)
                nc.tensor.matmul(om[:], XC[:, pos + sh: pos + sh + 128],
                                 WOF[:, t, :], start=(t == 0), stop=(t == 8))
            nc.scalar.activation(OMS[:, r, :], om[:], AF.Copy)
        # b_off (broadcast over rows)
        _bof = CON[:, 212:239]
        bof_b = bass.AP(tensor=_bof.tensor, offset=_bof.offset,
                        ap=[list(_bof.ap[0]), [0, RB], [1, 27]])
        nc.vector.tensor_tensor(OMS[:], OMS[:], bof_b, op=Alu.add)

        # ---- index / weight math ----
        SCR = mpool.tile([128, NS, RB, K2], DT.float32, tag="scr", name="scr")

        def s(i):
            return SCR[:, i]

        nc.scalar.activation(s(S_MSK), OMS[:, :, 18:27], AF.Sigmoid)

        offs = OMS[:, :, 0:18].rearrange("p r (k two) -> p two r k", two=2)
        dy, dx = offs[:, 0], offs[:, 1]

        def bc9(ap128x9):   # [128, 9] -> [128, RB, 9] broadcast over rows
            return bass.AP(tensor=ap128x9.tensor, offset=ap128x9.offset,
                           ap=[list(ap128x9.ap[0]), [0, RB], [1, 9]])

        def bcrow(ap128x64):  # [128, 64] row-consts -> [128, RB, 9] for batch bt
            sl = ap128x64[:, bt * RB:(bt + 1) * RB]
            return bass.AP(tensor=sl.tensor, offset=sl.offset,
                           ap=[list(sl.ap[0]), [1, RB], [0, 9]])

        KYb, KXb = bc9(KY), bc9(KX)
        HLb, HHb, HOFb = bcrow(HL), bcrow(HH), bcrow(HOF)
        v = nc.vector

        W4 = wpool.tile([128, 4, RB, K2], DT.float32, tag="w4")
        WR = wpool.tile([128, NG, K2, GG, 2, 8], DT.int16, tag="wr")

        v.tensor_tensor(s(S_TYS), dy, KYb, op=Alu.add)
        v.tensor_scalar(s(S_TYS), s(S_TYS), 0.0, None, Alu.max)
        # floor via the 2^23 magic number: RNE(x - 0.5) == floor(x) up to
        # integer ties, which bilinear continuity makes harmless
        v.tensor_scalar(s(S_Y0S), s(S_TYS), 8388607.5, 8388608.0,
                        Alu.add, Alu.subtract)
        v.tensor_tensor(s(S_WY), s(S_TYS), s(S_Y0S), op=Alu.subtract)
        v.tensor_scalar(s(S_Y1S), s(S_Y0S), 1.0, None, Alu.add)
        v.tensor_tensor(s(S_T0), s(S_Y0S), HLb, op=Alu.is_ge)
        v.tensor_tensor(s(S_T1), s(S_Y0S), HHb, op=Alu.is_le)
        v.tensor_tensor(s(S_V0), s(S_T0), s(S_T1), op=Alu.mult)
        v.tensor_tensor(s(S_T0), s(S_Y1S), HLb, op=Alu.is_ge)
        v.tensor_tensor(s(S_T1), s(S_Y1S), HHb, op=Alu.is_le)
        v.tensor_tensor(s(S_V1), s(S_T0), s(S_T1), op=Alu.mult)
        v.tensor_tensor(s(S_Y0C), s(S_Y0S), HLb, op=Alu.max)
        v.tensor_tensor(s(S_Y0C), s(S_Y0C), HHb, op=Alu.min)
        v.tensor_tensor(s(S_Y1C), s(S_Y1S), HLb, op=Alu.max)
        v.tensor_tensor(s(S_Y1C), s(S_Y1C), HHb, op=Alu.min)

        v.tensor_tensor(s(S_TXS), dx, KXb, op=Alu.add)
        v.tensor_scalar(s(S_TXS), s(S_TXS), 0.0, None, Alu.max)
        v.tensor_scalar(s(S_X0S), s(S_TXS), 8388607.5, 8388608.0,
                        Alu.add, Alu.subtract)
        v.tensor_tensor(s(S_WX), s(S_TXS), s(S_X0S), op=Alu.subtract)
        v.tensor_scalar(s(S_X1S), s(S_X0S), 1.0, None, Alu.add)
        v.tensor_scalar(s(S_XB), s(S_X0S), 16.0, None, Alu.max)
        v.tensor_scalar(s(S_XB), s(S_XB), 142.0, None, Alu.min)
        v.tensor_scalar(s(S_XB1), s(S_XB), 1.0, None, Alu.add)
        # slot weights: as_m = (1-wx)*[x0==xb+m] + wx*[x1==xb+m]
        v.tensor_scalar(s(S_AWX), s(S_WX), -1.0, 1.0, Alu.mult, Alu.add)
        v.tensor_tensor(s(S_T0), s(S_X0S), s(S_XB), op=Alu.is_equal)
        v.tensor_tensor(s(S_T1), s(S_X1S), s(S_XB), op=Alu.is_equal)
        v.tensor_tensor(s(S_T0), s(S_AWX), s(S_T0), op=Alu.mult)
        v.tensor_tensor(s(S_T1), s(S_WX), s(S_T1), op=Alu.mult)
        v.tensor_tensor(s(S_AS0), s(S_T0), s(S_T1), op=Alu.add)
        v.tensor_tensor(s(S_T0), s(S_X0S), s(S_XB1), op=Alu.is_equal)
        v.tensor_tensor(s(S_T1), s(S_X1S), s(S_XB1), op=Alu.is_equal)
        v.tensor_tensor(s(S_T0), s(S_AWX), s(S_T0), op=Alu.mult)
        v.tensor_tensor(s(S_T1), s(S_WX), s(S_T1), op=Alu.mult)
        v.tensor_tensor(s(S_AS1), s(S_T0), s(S_T1), op=Alu.add)
        # y weights with validity and mask folded in
        v.tensor_scalar(s(S_AWY), s(S_WY), -1.0, 1.0, Alu.mult, Alu.add)
        v.tensor_tensor(s(S_WY0M), s(S_AWY), s(S_V0), op=Alu.mult)
        v.tensor_tensor(s(S_WY0M), s(S_WY0M), s(S_MSK), op=Alu.mult)
        v.tensor_tensor(s(S_WY1M), s(S_WY), s(S_V1), op=Alu.mult)
        v.tensor_tensor(s(S_WY1M), s(S_WY1M), s(S_MSK), op=Alu.mult)
        v.tensor_tensor(W4[:, 0], s(S_WY0M), s(S_AS0), op=Alu.mult)
        v.tensor_tensor(W4[:, 1], s(S_WY0M), s(S_AS1), op=Alu.mult)
        v.tensor_tensor(W4[:, 2], s(S_WY1M), s(S_AS0), op=Alu.mult)
        v.tensor_tensor(W4[:, 3], s(S_WY1M), s(S_AS1), op=Alu.mult)
        # gather unit index = y0c*128 + (xb + 128*h - 2064)
        v.tensor_tensor(s(S_ADR), s(S_XB), HOFb, op=Alu.add)
        v.scalar_tensor_tensor(s(S_I0F), s(S_Y0C), 128.0, s(S_ADR),
                               Alu.mult, Alu.add)
        v.scalar_tensor_tensor(s(S_I1F), s(S_Y1C), 128.0, s(S_ADR),
                               Alu.mult, Alu.add)

        # ---- wrap indices into the 16-partition gather layout ----
        # PE-transpose the f32 idx region through the permuted identity
        # (j -> x = (j%8)*16 + j//8) with m = (klow, rr, c2), convert to
        # int16, then per-(i, g) DMAs whose destination is one contiguous
        # 1024-element run in WR (16B descriptor runs instead of 2B).
        T3A = tpool.tile([128, NG, 128], DT.int16, tag="t3a")
        T3B = tpool.tile([128, NG, 128], DT.int16, tag="t3b")  # [0:16] used
        ibase = s(S_I0F)   # slots S_I0F/S_I1F adjacent: [128, 2, RB, 9]
        for g in range(NG):
            off0 = ibase.offset + g * GG * 9
            inA = bass.AP(tensor=ibase.tensor, offset=off0,
                          ap=[list(ibase.ap[0]), [1, 8], [9, GG], [RB * 9, 2]])
            inB = bass.AP(tensor=ibase.tensor, offset=off0 + 8,
                          ap=[list(ibase.ap[0]), [9, GG], [RB * 9, 2]])
            MA = (K2 - 1) * GG * 2
            MB = GG * 2
            SA = tpool.tile([128, MA], DT.float32, tag="sa")
            v.tensor_copy(SA[:], inA)
            SB = tpool.tile([128, MB], DT.float32, tag="sb")
            v.tensor_copy(SB[:], inB)
            TP = tp_ps.tile([128, 128], DT.float32, tag="tp", name="tp")
            nc.tensor.matmul(TP[0:MA], SA[:], IDNP[:], start=True, stop=True)
            v.tensor_copy(T3A[0:MA, g], TP[0:MA])
            TP2 = tp_ps.tile([128, 128], DT.float32, tag="tp", name="tp")
            nc.tensor.matmul(TP2[0:MB], SB[:], IDNP[:], start=True, stop=True)
            v.tensor_copy(T3B[0:MB, g], TP2[0:MB])
        # bounce via DRAM: a DRAM source has no partition-order constraint,
        # so one load per (chunk, g) fills the 16-partition wrap block with
        # 16B-contiguous descriptor runs.
        MA = (K2 - 1) * GG * 2
        MB = GG * 2
        aoff = bt * 128 * NG * 128
        boff = bt * 16 * NG * 128
        nc.sync.dma_start(bass.AP(tensor=wrscr, offset=aoff,
                                  ap=[[NG * 128, MA], [1, NG * 128]]),
                          T3A[0:MA])
        nc.sync.dma_start(bass.AP(tensor=wrscrb, offset=boff,
                                  ap=[[NG * 128, MB], [1, NG * 128]]),
                          T3B[0:MB])
        GW = K2 * GG * 2 * 8   # WR elements per (partition, g)
        MA = (K2 - 1) * GG * 2
        MB = GG * 2
        dstp = WR[0:16]
        for g in range(NG):
            dstA = bass.AP(tensor=dstp.tensor, offset=dstp.offset + g * GW,
                           ap=[list(dstp.ap[0]), [1, MA * 8]])
            srcA = bass.AP(tensor=wrscr, offset=aoff + g * 128,
                           ap=[[8, 16], [NG * 128, MA], [1, 8]])
            nc.sync.dma_start(dstA, srcA)
            dstB = bass.AP(tensor=dstp.tensor,
                           offset=dstp.offset + g * GW + MA * 8,
                           ap=[list(dstp.ap[0]), [1, MB * 8]])
            srcB = bass.AP(tensor=wrscrb, offset=boff + g * 128,
                           ap=[[8, 16], [NG * 128, MB], [1, 8]])
            nc.sync.dma_start(dstB, srcB)
        for gc in range(1, 8):
            nc.sync.dma_start(WR[16 * gc:16 * (gc + 1)], WR[0:16])

        # ---- gather + weighted-transpose MAC on PE + pointwise ----
        for gg in range(NG):
            TRS = trs_ps.tile([128, GG, 128], DT.float32, tag="trs",
                              name="trs")
            for k in range(K2):
                GT = gpool.tile([128, 2 * GG, 256], DT.float16, tag="gt",
                                name="gt")
                src = bass.AP(tensor=xg, offset=k * IMG_U * 128,
                              ap=[[128, 16384], [1, 256]])
                idxs = WR[:, gg, k].rearrange("p r c s -> p (r c s)")
                nc.gpsimd.dma_gather(GT[:], src, idxs, NIDX, nidx_reg, 256,
                                     elem_step=128, queue_num=gidx[0] % 4)
                gidx[0] += 1
                DK = dpool.tile([128, GG, 4, 128], DT.float16, tag="d")
                for rr in range(GG):
                    rb = gg * GG + rr
                    for c2 in range(2):
                        for s2 in range(2):
                            wsc = W4[:, c2 * 2 + s2, rb, k:k + 1]
                            v.tensor_scalar(DK[:, rr, c2 * 2 + s2], IDN[:],
                                            wsc, None, Alu.mult)
                for rr in range(GG):
                    for c2 in range(2):
                        for s2 in range(2):
                            g = GT[:, rr * 2 + c2, s2 * 128:(s2 + 1) * 128]
                            # psum zero regions are 2KB = 4 rows of TRS;
                            # one accumulation group per region
                            nc.tensor.matmul(
                                TRS[:, rr], g, DK[:, rr, c2 * 2 + s2],
                                start=(k == 0 and rr % 4 == 0 and c2 == 0
                                       and s2 == 0),
                                stop=(k == 8 and rr % 4 == 3 and c2 == 1
                                      and s2 == 1))
            OUTS = opool.tile([128, 2, GG, 128], DT.float16, tag="outs")
            for rr in range(GG):
                RT = rpool.tile([128, 128], DT.float16, tag="rt")
                nc.scalar.activation(RT[:], TRS[:, rr], AF.Copy)
                for oh in range(2):
                    PW = pw_ps.tile([128, 128], DT.float32, tag="pw",
                                    name="pw")
                    nc.tensor.matmul(PW[:], WPW[:, oh * 128:(oh + 1) * 128],
                                     RT[:], start=True, stop=True)
                    nc.scalar.activation(OUTS[:, oh, rr, :], PW[:], AF.Identity,
                                         bias=BPW[:, oh:oh + 1])
            r0 = bt * RB + gg * GG
            nc.sync.dma_start(out_v[:, :, r0:r0 + GG, :], OUTS[:])


# ---------------- host side ----------------

def host_prep(inputs):
    x = np.asarray(inputs["x"], np.float32)
    w_off = np.asarray(inputs["w_off"], np.float32)
    b_off = np.asarray(inputs["b_off"], np.float32)
    w_dw = np.asarray(inputs["w_dw"], np.float32)
    b_dw = np.asarray(inputs["b_dw"], np.float32)
    w_pw = np.asarray(inputs["w_pw"], np.float32)
    b_pw = np.asarray(inputs["b_pw"], np.float32)

    wk = w_dw.reshape(C, K2)
    woff_p = np.ascontiguousarray(
        w_off.transpose(1, 2, 3, 0).reshape(C, K2 * 27)).astype(np.float16)
    wpw_p = np.ascontiguousarray(w_pw.T).astype(np.float16)
    idn = np.eye(128, dtype=np.float16)
    idnp = np.zeros((128, 128), np.float32)
    jj = np.arange(128)
    idnp[(jj % 8) * 16 + jj // 8, jj] = 1.0
    bpw_eff = (b_pw + w_pw @ b_dw).astype(np.float32)

    ky = (np.arange(K2) // 3 - 1).astype(np.float32)
    kx = (np.arange(K2) % 3 - 1).astype(np.float32)

    xgs = []
    for b in range(B):
        flat = np.ascontiguousarray(x[b].transpose(1, 2, 0).reshape(H * W, C))
        img = np.zeros([K2, IMG_U, 128], np.float16)
        for k in range(K2):
            img[k, :H * W] = flat * wk[:, k][None, :]
        xgs.append(img)

    in_maps = []
    for core in range(8):
        b, half = core // 2, core % 2
        r0 = half * ROWS
        xcp = np.zeros([C, 66, 130], np.float32)
        lo, hi = max(r0 - 1, 0), min(r0 + 65, H)
        xcp[:, lo - (r0 - 1): hi - (r0 - 1), 1:129] = x[b][:, lo:hi, :]
        xcp = xcp.astype(np.float16).reshape(C, 66 * 130)

        hvec = (r0 + np.arange(ROWS)).astype(np.float32)
        cons = np.zeros([128, CONS_W], np.float32)
        cons[:, 0:9] = ky[None, :] + 16.0
        cons[:, 9:18] = kx[None, :] + 16.0 + np.arange(128, dtype=np.float32)[:, None]
        cons[:, 18:82] = (16.0 - hvec)[None, :]
        cons[:, 82:146] = (143.0 - hvec)[None, :]
        cons[:, 146:210] = (128.0 * hvec - 2064.0)[None, :]
        cons[:, 210:212] = bpw_eff.reshape(2, 128).T
        cons[:, 212:239] = b_off[None, :]

        in_maps.append({
            "xc": xcp, "xg": xgs[b], "woff": woff_p, "wpw": wpw_p,
            "idn": idn, "idnp": idnp, "cons": cons,
        })
    return in_maps


def assemble(results):
    out = np.zeros([B, O, H, W], np.float32)
    for core, r in enumerate(results):
        b, half = core // 2, core % 2
        out[b, :, half * ROWS:(half + 1) * ROWS, :] = r["out"].astype(np.float32)
    return out


# ---- single-sync-wait legalization (inlined) ----
_doc = """Legalize BIR for walrus builds that allow only ONE sync wait per
instruction: hoist extra waits onto same-engine NOPs inserted immediately
before the offending instruction."""
import copy

def _make_nop(nc, engine):
    nop = nc.engines[engine].nop(nofuse=True).ins
    # the builder appended it to nc.cur_bb; steal it from wherever it landed
    for f in nc.m.functions:
        for bb in f.blocks:
            il = bb.instructions
            if il and il[-1].name == nop.name:
                il.pop()
                bb.instructions = il
                return nop
    raise RuntimeError("freshly built nop not found")

def legalize_single_wait(nc):
    n_split = 0
    for f in nc.m.functions:
        for bb in f.blocks:
            insts = bb.instructions
            if not any(i.sync_info and len(i.sync_info.on_wait) > 1 for i in insts):
                continue
            out = []
            for inst in insts:
                si = inst.sync_info
                if si and len(si.on_wait) > 1:
                    waits = list(si.on_wait)
                    for w in waits[:-1]:
                        nop = _make_nop(nc, inst.engine)
                        nsi = copy.deepcopy(si)
                        nsi.on_wait = [w]
                        nsi.on_update = []
                        nop.sync_info = nsi
                        out.append(nop)
                    si.on_wait = [waits[-1]]
                    n_split += 1
                out.append(inst)
            bb.instructions = out
    return n_split


_CACHED_NC = None


def kernel(**inputs):
    global _CACHED_NC
    from concourse import bass_utils
    in_maps = host_prep(inputs)
    if _CACHED_NC is None:
        _CACHED_NC = build_nc()
    res = bass_utils.run_bass_kernel_spmd(_CACHED_NC, in_maps,
                                          core_ids=list(range(8)))
    return assemble(res.results)
